# revision 27
# baseline (speedup 1.0000x reference)
"""Self-contained Trainium2 Bass kernel for 12-head attention.

Module: out = softmax((xq Wq^T)(xk Wk^T)^T / sqrt(64)) (xv Wv^T) Wp^T + bp
Shapes: xq/xk/xv [2, 2048, 768]; W* [768, 768]; bp [768].

Sharding (8 cores): core c handles batch b = c//4 and head group g = c%4
(3 of the 12 heads).  Each core computes its heads' attention plus the
partial output projection (contraction over its 192 feature columns of
Wp).  Host unshard: out[b] = sum of the 4 group partials + bias.

Per-core dataflow (all matmul operands bf16, fp32 PSUM accumulation):
  Qt/Kt [64, 2048] transposed layout, head pairs packed into 128
  partitions; V [2048, 64] natural + ones column (denominator trick).
  S^T[k,q] = Kt^T Qt via 64-row PE array tiling (two concurrent tiles);
  Et = exp(S*scale) on ScalarE (|S*scale| <= ~3, no max needed);
  Ot[65,q] = V_aug^T Et row-tiled into two PSUM banks; normalize via
  reciprocal + rank-1 PE broadcast; Y[q,768] = OtN^T WpT per q-chunk.
"""

import os
import sys

import numpy as np

for _p in ("/opt/trn_rl_repo",):
    if _p not in sys.path and os.path.isdir(_p):
        sys.path.insert(0, _p)

import ml_dtypes

DIM = 768
NH = 12
HD = 64
N = 2048
B = 2
SCALE = HD ** -0.5
NCORES = 8
HPG = 3  # heads per group (core)

_BUILT = {}
LAST_RESULT = None


def build_bass():
    import concourse.bacc as bacc
    import concourse.mybir as mybir
    import concourse.tile as tile

    bf16 = mybir.dt.bfloat16
    f32 = mybir.dt.float32
    AF = mybir.ActivationFunctionType

    nc = bacc.Bacc("TRN2", target_bir_lowering=False, debug=False)
    # x tensors ship strip-tiled: [KC=6, NSTRIP=4, 128, 512] so each
    # [128, 512] strip is one contiguous 128KB DMA read.
    xqT = nc.declare_dram_parameter("xqT", [24, 128, 512], bf16, isOutput=False)
    xkT = nc.declare_dram_parameter("xkT", [24, 128, 512], bf16, isOutput=False)
    xvT = nc.declare_dram_parameter("xvT", [24, 128, 512], bf16, isOutput=False)
    wqkT = nc.declare_dram_parameter("wqkT", [DIM, 2 * 192], bf16, isOutput=False)
    wvT = nc.declare_dram_parameter("wvT", [DIM, 192], bf16, isOutput=False)
    wpT = nc.declare_dram_parameter("wpT", [192, DIM], bf16, isOutput=False)
    out = nc.declare_dram_parameter("out", [N, DIM], f32, isOutput=True)

    KC = DIM // 128  # 6 contraction chunks for projections
    QB = N // 512    # 4 query blocks
    SEQC = N // 128  # 16 sequence chunks

    with tile.TileContext(nc) as tc:
        from collections import deque
        from contextlib import ExitStack

        with ExitStack() as ctx:
            pX = ctx.enter_context(tc.tile_pool(name="px", bufs=1))
            pW = ctx.enter_context(tc.tile_pool(name="pw", bufs=1))
            pP = ctx.enter_context(tc.tile_pool(name="pp", bufs=1))
            pEt = ctx.enter_context(tc.tile_pool(name="pet", bufs=6))
            pSm = ctx.enter_context(tc.tile_pool(name="psm", bufs=4))
            pY = ctx.enter_context(tc.tile_pool(name="py", bufs=3))
            psS2 = ctx.enter_context(tc.tile_pool(name="pss2", bufs=2, space="PSUM"))
            psO = ctx.enter_context(tc.tile_pool(name="pso", bufs=4, space="PSUM"))

            # ---------------- DMA inputs (first-needed first) ------------
            tiles = {}
            for nm, src_t, w in (
                ("wqk", wqkT, 384),
                ("xk", xkT, N),
                ("xq", xqT, N),
                ("wv", wvT, 192),
                ("xv", xvT, N),
            ):
                for k in range(KC):
                    t = pX.tile([128, w], bf16, tag=f"{nm}{k}", name=f"{nm}{k}")
                    if w == N:
                        for j in range(4):
                            nc.sync.dma_start(
                                t[:, 512 * j : 512 * (j + 1)],
                                src_t[4 * k + j, :, :],
                            )
                    else:
                        nc.sync.dma_start(
                            t[:], src_t[128 * k : 128 * (k + 1), :]
                        )
                    tiles[nm, k] = t
            xq_t = [tiles["xq", k] for k in range(KC)]
            xk_t = [tiles["xk", k] for k in range(KC)]
            xv_t = [tiles["xv", k] for k in range(KC)]
            wqk_t = [tiles["wqk", k] for k in range(KC)]
            wv_t = [tiles["wv", k] for k in range(KC)]
            wp_t = []
            for h in range(HPG):
                t = pW.tile([128, DIM], bf16, tag=f"wp{h}", name=f"wp{h}")
                nc.sync.dma_start(t[0:64, :], wpT[64 * h : 64 * (h + 1), :])
                wp_t.append(t)
            ones_t = pW.tile([128, 64], bf16, tag="ones")
            nc.gpsimd.memset(ones_t[:], 1.0)

            # ------------- projections (all outputs at partitions 0-63) --
            # qk[h] = [Qt_h; Kt_h] packed? No: one tile per head per kind,
            # data at partitions 0-63 so full-mode K=64 matmuls can read it.
            qt = [pP.tile([128, N], bf16, tag=f"qt{h}", name=f"qt{h}") for h in range(3)]
            kt = [pP.tile([128, N], bf16, tag=f"kt{h}", name=f"kt{h}") for h in range(3)]

            def qk_group(qb, wc, xt, dst0, dst1):
                qs = slice(512 * qb, 512 * (qb + 1))
                ps = psO.tile([128, 512], f32, tag="o", name="ps_qk")
                for k in range(KC):
                    nc.tensor.matmul(
                        ps[:],
                        lhsT=wqk_t[k][:, wc : wc + 128],
                        rhs=xt[k][:, qs],
                        start=(k == 0),
                        stop=(k == KC - 1),
                    )
                nc.vector.tensor_copy(dst0[0:64, qs], ps[0:64, :])
                nc.vector.tensor_copy(dst1[64:128, qs], ps[64:128, :])
                # head-1 rows also needed at partitions 0-63 (PV rhs is full
                # 128 but S row-tile T8 reads 64-127; out-proj needs 0-63):
                # actually S T8 reads 64-127 directly; no shift needed here.

            def g3_group(qb):
                qs = slice(512 * qb, 512 * (qb + 1))
                ps = psO.tile([128, 512], f32, tag="o", name="ps_g3")
                for k in range(KC):
                    nc.tensor.matmul(
                        ps[0:64, :],
                        lhsT=wqk_t[k][:, 128:192],
                        rhs=xq_t[k][:, qs],
                        start=(k == 0),
                        stop=(k == KC - 1),
                        tile_position=(0, 0),
                        skip_group_check=True,
                    )
                    nc.tensor.matmul(
                        ps[64:128, :],
                        lhsT=wqk_t[k][:, 320:384],
                        rhs=xk_t[k][:, qs],
                        start=(k == 0),
                        stop=(k == KC - 1),
                        tile_position=(0, 64),
                        skip_group_check=True,
                    )
                nc.vector.tensor_copy(qt[2][0:64, qs], ps[0:64, :])
                nc.vector.tensor_copy(kt[2][64:128, qs], ps[64:128, :])
                nc.gpsimd.dma_start(kt[2][0:64, qs], kt[2][64:128, qs])
                nc.gpsimd.dma_start(qt[2][64:128, qs], qt[2][0:64, qs])

            v_t = []
            for sc in range(SEQC):
                v_t.append(
                    pP.tile([128, 3 * 65], bf16, tag=f"v{sc}", name=f"v{sc}")
                )

            def v_group(sc):
                ps = psO.tile([128, 512], f32, tag="o", name="ps_v")
                for k in range(KC):
                    nc.tensor.matmul(
                        ps[:, 0:192],
                        lhsT=xv_t[k][:, 128 * sc : 128 * (sc + 1)],
                        rhs=wv_t[k][:],
                        start=(k == 0),
                        stop=(k == KC - 1),
                    )
                vt = v_t[sc]
                for h in range(HPG):
                    nc.vector.tensor_copy(
                        vt[:, 65 * h : 65 * h + 64], ps[:, 64 * h : 64 * (h + 1)]
                    )
                    nc.gpsimd.memset(vt[:, 65 * h + 64 : 65 * h + 65], 1.0)

            # K-side first: attention S needs all of kt before step 0;
            # V before g3 (V is consumed from attention step 1, g3 at the
            # first U1 window ~16 steps in).
            for qb in range(QB):
                qk_group(qb, 192, xk_t, kt[0], kt[1])
            for qb in range(QB):
                qk_group(qb, 0, xq_t, qt[0], qt[1])
            for qb in range(QB):
                g3_group(qb)
            for sc in range(SEQC):
                v_group(sc)

            # ---------------- attention ----------------
            otn = []
            for h in range(HPG):
                otn.append(pP.tile([128, N], bf16, tag=f"otn{h}", name=f"otn{h}"))
            tstash = []
            for i in range(12):
                tstash.append(
                    pSm.tile([65, 512], bf16, tag=f"tmp{i}", bufs=1, name=f"tmp{i}")
                )
            # denominators packed partition-major: head (qb,h) occupies a
            # [128, 4] column block; reciprocal then costs ~12 free-elems.
            denP = pSm.tile([128, 48], f32, tag="denP", bufs=1, name="denP")
            recP = pSm.tile([128, 48], bf16, tag="recP", bufs=1, name="recP")
            gstep = [0]
            bg = deque()  # entries: (earliest_step, closure)


            def pump(k):
                for _ in range(k):
                    if bg and bg[0][0] <= gstep[0]:
                        bg.popleft()[1]()

            def collect(h, qb, o_ps):
                idx = 3 * qb + h
                nc.vector.tensor_copy(tstash[idx][:], o_ps[0:65, :])
                # denominator stays f32: DMA the psum row is impossible, so
                # re-read the bf16 stash row would lose bits; instead copy
                # the f32 denom row via ScalarE into the packed tile? DMA
                # can't read PSUM. Keep a small f32 row copy on DVE:
                nc.gpsimd.dma_start(
                    denP[:, 4 * idx : 4 * idx + 4],
                    tstash[idx][64:65, :],
                )

            def push_finalize_outproj(qb):
                qs = slice(512 * qb, 512 * (qb + 1))
                st = {}
                s0 = gstep[0]

                def f_recip():
                    cs = slice(12 * qb, 12 * qb + 12)
                    with nc.allow_low_precision(reason="softmax denom bcast"):
                        nc.vector.reciprocal(recP[:, cs], denP[:, cs])

                bg.append((s0 + 1, f_recip))

                def mk_head(h):
                    def f_head():
                        idx = 3 * qb + h
                        r1 = pSm.tile([1, 512], bf16, tag="r1", name="r1")
                        nc.gpsimd.dma_start(
                            r1[:], recP[:, 4 * idx : 4 * idx + 4]
                        )
                        rbs = pSm.tile([64, 512], bf16, tag="rbs", name="rbs")
                        nc.gpsimd.partition_broadcast(rbs[:], r1[0:1, :])
                        for j in range(4):
                            js = slice(128 * j, 128 * (j + 1))
                            gs = slice(512 * qb + 128 * j, 512 * qb + 128 * (j + 1))
                            nc.vector.tensor_mul(
                                otn[h][0:64, gs], tstash[idx][0:64, js], rbs[:, js]
                            )
                            if h == 1:
                                nc.gpsimd.dma_start(
                                    otn[0][64:128, gs], otn[1][0:64, gs]
                                )

                    return f_head

                for h in range(HPG):
                    bg.append((s0 + 6 + 2 * h, mk_head(h)))

                def mk_mm(qc, off, w, first, K128):
                    def f_mm():
                        if first:
                            st[qc] = (
                                psO.tile([128, 512], f32, tag="o", name="y1"),
                                pY.tile([128, DIM], f32, tag="y", name="ysb"),
                            )
                        y_ps, _ = st[qc]
                        cs = slice(128 * qc, 128 * (qc + 1))
                        if K128:
                            nc.tensor.matmul(
                                y_ps[:, 0:w],
                                lhsT=otn[0][:, cs],
                                rhs=wp_t[0][:, off : off + w],
                                start=True,
                                stop=False,
                            )
                        else:
                            nc.tensor.matmul(
                                y_ps[:, 0:w],
                                lhsT=otn[2][0:64, cs],
                                rhs=wp_t[2][0:64, off : off + w],
                                start=False,
                                stop=True,
                            )

                    return f_mm

                def mk_evac(qc, off, w, last):
                    def f_evac():
                        y_ps, y_sb = st[qc]
                        nc.vector.tensor_copy(y_sb[:, off : off + w], y_ps[:, 0:w])
                        if last:
                            cs = slice(128 * qc, 128 * (qc + 1))
                            nc.sync.dma_start(out[cs, :], y_sb[:])

                    return f_evac

                d = s0 + 13
                for qc in range(4 * qb, 4 * qb + 4):
                    bg.append((d, mk_mm(qc, 0, 512, True, True)))
                    bg.append((d, mk_mm(qc, 0, 512, False, False)))
                    bg.append((d + 1, mk_evac(qc, 0, 512, False)))
                    bg.append((d + 1, mk_mm(qc, 512, 256, False, True)))
                    bg.append((d + 2, mk_mm(qc, 512, 256, False, False)))
                    bg.append((d + 2, mk_evac(qc, 512, 256, True)))
                    d += 2

            # wp_t[0] must hold heads 0 and 1 stacked for the K=128 path
            nc.sync.dma_start(wp_t[0][64:128, :], wpT[64:128, :])

            pend = None

            def emit_pv(et, u, qb, step, o_ps):
                for c in (0, 1):
                    cs = slice(512 * c, 512 * (c + 1))
                    if u == 0:
                        hh, kc, oi = c, step, c
                    else:
                        hh, kc, oi = 2, 2 * step + c, 0
                    vsl = slice(65 * hh, 65 * hh + 65)
                    nc.tensor.matmul(
                        o_ps[oi][0:65, :],
                        lhsT=v_t[kc][:, vsl],
                        rhs=et[:, cs],
                        start=(kc == 0),
                        stop=(kc == SEQC - 1),
                    )

            windows = []
            for qb in range(QB):
                windows.append((0, qb))
                windows.append((1, qb))

            for u, qb in windows:
                qs = slice(512 * qb, 512 * (qb + 1))
                o_ps = {}
                for i in range(2 if u == 0 else 1):
                    o_ps[i] = psO.tile([128, 512], f32, tag="o", name=f"o{i}")
                nsteps = SEQC if u == 0 else SEQC // 2
                for step in range(nsteps):
                    s2 = psS2.tile([128, 1024], f32, tag="s2", name="s2")
                    if u == 0:
                        # S row-tile pair: head0 on rows 0-63, head1 on 64-127
                        nc.tensor.matmul(
                            s2[:, 0:512],
                            lhsT=kt[0][0:64, 128 * step : 128 * (step + 1)],
                            rhs=qt[0][0:64, qs],
                            start=True,
                            stop=True,
                            tile_position=(0, 0),
                            skip_group_check=True,
                        )
                        nc.tensor.matmul(
                            s2[:, 512:1024],
                            lhsT=kt[1][64:128, 128 * step : 128 * (step + 1)],
                            rhs=qt[1][64:128, qs],
                            start=True,
                            stop=True,
                            tile_position=(64, 0),
                            skip_group_check=True,
                        )
                    else:
                        k0, k1 = 2 * step, 2 * step + 1
                        nc.tensor.matmul(
                            s2[:, 0:512],
                            lhsT=kt[2][0:64, 128 * k0 : 128 * (k0 + 1)],
                            rhs=qt[2][0:64, qs],
                            start=True,
                            stop=True,
                            tile_position=(0, 0),
                            skip_group_check=True,
                        )
                        nc.tensor.matmul(
                            s2[:, 512:1024],
                            lhsT=kt[2][64:128, 128 * k1 : 128 * (k1 + 1)],
                            rhs=qt[2][64:128, qs],
                            start=True,
                            stop=True,
                            tile_position=(64, 0),
                            skip_group_check=True,
                        )
                    et = pEt.tile([128, 1024], bf16, tag="et", name="et")
                    nc.scalar.activation(et[:], s2[:], AF.Exp, scale=SCALE)
                    if pend is not None:
                        emit_pv(*pend)
                        if pend[3] == (SEQC if pend[1] == 0 else SEQC // 2) - 1:
                            pu, pqb, po = pend[1], pend[2], pend[4]
                            if pu == 0:
                                collect(0, pqb, po[0])
                                collect(1, pqb, po[1])
                            else:
                                collect(2, pqb, po[0])
                                push_finalize_outproj(pqb)
                    pend = (et, u, qb, step, o_ps)
                    gstep[0] += 1
                    pump(2 if len(bg) > 16 else 1)
            emit_pv(*pend)
            collect(2, QB - 1, pend[4][0])
            push_finalize_outproj(QB - 1)
            gstep[0] += 100000
            pump(len(bg))

    nc.compile()
    return nc


def _shard_inputs(xq, xk, xv, Wq, Wk, Wv, Wp):
    bf = ml_dtypes.bfloat16
    in_maps = []
    def tile_x(a):
        # [N, DIM] -> transpose -> [DIM, N] -> [6, 128, 4, 512] -> [24, 128, 512]
        t = a.T.reshape(6, 128, 4, 512).transpose(0, 2, 1, 3).reshape(24, 128, 512)
        return np.ascontiguousarray(t).astype(bf)

    xT = {}
    for b in range(B):
        xT[b] = tuple(tile_x(a[b]) for a in (xq, xk, xv))
    for c in range(NCORES):
        b, g = c // 4, c % 4
        hs = slice(192 * g, 192 * (g + 1))
        wqk = np.concatenate([Wq[hs].T, Wk[hs].T], axis=1)
        in_maps.append(
            {
                "xqT": xT[b][0],
                "xkT": xT[b][1],
                "xvT": xT[b][2],
                "wqkT": np.ascontiguousarray(wqk).astype(bf),
                "wvT": np.ascontiguousarray(Wv[hs].T).astype(bf),
                "wpT": np.ascontiguousarray(Wp[:, hs].T).astype(bf),
            }
        )
    return in_maps


def _ensure_ntff_hook():
    """Register the axon NTFF profiling hook if the stub antenv lacks it."""
    import types

    try:
        from antenv.axon_hooks import get_axon_ntff_profile_hook  # noqa: F401

        return
    except ImportError:
        pass
    try:
        import antenv
        from trn_agent_boot.trn_boot import _ntff_profile_via_ctypes

        so_path = "/opt/axon/libaxon_pjrt.so"
        hook = _ntff_profile_via_ctypes(so_path) if os.path.exists(so_path) else None
        mod = types.ModuleType("antenv.axon_hooks")
        _state = {"h": hook}
        mod.get_axon_ntff_profile_hook = lambda: _state["h"]
        mod.set_axon_ntff_profile_hook = lambda h: _state.__setitem__("h", h)
        sys.modules["antenv.axon_hooks"] = mod
        antenv.axon_hooks = mod
    except Exception:
        pass


def kernel(xq, xk, xv, Wq, Wk, Wv, Wp, bp):
    global LAST_RESULT
    from concourse.bass_utils import run_bass_kernel_spmd

    key = "nc"
    if key not in _BUILT:
        _BUILT[key] = build_bass()
    nc = _BUILT[key]

    xq, xk, xv = (np.asarray(a, np.float32) for a in (xq, xk, xv))
    Wq, Wk, Wv, Wp = (np.asarray(a, np.float32) for a in (Wq, Wk, Wv, Wp))
    bp = np.asarray(bp, np.float32)

    in_maps = _shard_inputs(xq, xk, xv, Wq, Wk, Wv, Wp)
    trace = bool(os.environ.get("BASS_KERNEL_TRACE"))
    if trace:
        _ensure_ntff_hook()
    res = run_bass_kernel_spmd(
        nc, in_maps, core_ids=list(range(NCORES)), trace=trace
    )
    LAST_RESULT = res
    parts = [res.results[i]["out"].astype(np.float32) for i in range(NCORES)]
    out = np.stack(
        [
            parts[0] + parts[1] + parts[2] + parts[3],
            parts[4] + parts[5] + parts[6] + parts[7],
        ]
    )
    return (out + bp[None, None, :]).astype(np.float32)


# revision 28
# speedup vs baseline: 1.0479x; 1.0479x over previous
"""Self-contained Trainium2 Bass kernel for 12-head attention.

Module: out = softmax((xq Wq^T)(xk Wk^T)^T / sqrt(64)) (xv Wv^T) Wp^T + bp
Shapes: xq/xk/xv [2, 2048, 768]; W* [768, 768]; bp [768].

Sharding (8 cores): core c handles batch b = c//4 and head group g = c%4
(3 of the 12 heads).  Each core computes its heads' attention plus the
partial output projection (contraction over its 192 feature columns of
Wp).  Host unshard: out[b] = sum of the 4 group partials + bias.

Per-core dataflow (all matmul operands bf16, fp32 PSUM accumulation):
  Qt/Kt [64, 2048] transposed layout, head pairs packed into 128
  partitions; V [2048, 64] natural + ones column (denominator trick).
  S^T[k,q] = Kt^T Qt via 64-row PE array tiling (two concurrent tiles);
  Et = exp(S*scale) on ScalarE (|S*scale| <= ~3, no max needed);
  Ot[65,q] = V_aug^T Et row-tiled into two PSUM banks; normalize via
  reciprocal + rank-1 PE broadcast; Y[q,768] = OtN^T WpT per q-chunk.
"""

import os
import sys

import numpy as np

for _p in ("/opt/trn_rl_repo",):
    if _p not in sys.path and os.path.isdir(_p):
        sys.path.insert(0, _p)

import ml_dtypes

DIM = 768
NH = 12
HD = 64
N = 2048
B = 2
SCALE = HD ** -0.5
NCORES = 8
HPG = 3  # heads per group (core)

_BUILT = {}
LAST_RESULT = None


def build_bass():
    import concourse.bacc as bacc
    import concourse.mybir as mybir
    import concourse.tile as tile

    bf16 = mybir.dt.bfloat16
    f32 = mybir.dt.float32
    AF = mybir.ActivationFunctionType

    nc = bacc.Bacc("TRN2", target_bir_lowering=False, debug=False)
    xqT = nc.declare_dram_parameter("xqT", [DIM, N], bf16, isOutput=False)
    xkT = nc.declare_dram_parameter("xkT", [DIM, N], bf16, isOutput=False)
    xvT = nc.declare_dram_parameter("xvT", [DIM, N], bf16, isOutput=False)
    wqkT = nc.declare_dram_parameter("wqkT", [DIM, 2 * 192], bf16, isOutput=False)
    wvT = nc.declare_dram_parameter("wvT", [DIM, 192], bf16, isOutput=False)
    wpT = nc.declare_dram_parameter("wpT", [192, DIM], bf16, isOutput=False)
    out = nc.declare_dram_parameter("out", [N, DIM], f32, isOutput=True)

    KC = DIM // 128  # 6 contraction chunks for projections
    QB = N // 512    # 4 query blocks
    SEQC = N // 128  # 16 sequence chunks

    with tile.TileContext(nc) as tc:
        from collections import deque
        from contextlib import ExitStack

        with ExitStack() as ctx:
            pX = ctx.enter_context(tc.tile_pool(name="px", bufs=1))
            pW = ctx.enter_context(tc.tile_pool(name="pw", bufs=1))
            pP = ctx.enter_context(tc.tile_pool(name="pp", bufs=1))
            pEt = ctx.enter_context(tc.tile_pool(name="pet", bufs=6))
            pSm = ctx.enter_context(tc.tile_pool(name="psm", bufs=4))
            pY = ctx.enter_context(tc.tile_pool(name="py", bufs=3))
            psS2 = ctx.enter_context(tc.tile_pool(name="pss2", bufs=2, space="PSUM"))
            psO = ctx.enter_context(tc.tile_pool(name="pso", bufs=4, space="PSUM"))

            # ---------------- DMA inputs (first-needed first) ------------
            tiles = {}
            for nm, src_t, w in (
                ("wqk", wqkT, 384),
                ("xk", xkT, N),
                ("xq", xqT, N),
                ("wv", wvT, 192),
                ("xv", xvT, N),
            ):
                for k in range(KC):
                    t = pX.tile([128, w], bf16, tag=f"{nm}{k}", name=f"{nm}{k}")
                    nc.sync.dma_start(t[:], src_t[128 * k : 128 * (k + 1), :])
                    tiles[nm, k] = t
            xq_t = [tiles["xq", k] for k in range(KC)]
            xk_t = [tiles["xk", k] for k in range(KC)]
            xv_t = [tiles["xv", k] for k in range(KC)]
            wqk_t = [tiles["wqk", k] for k in range(KC)]
            wv_t = [tiles["wv", k] for k in range(KC)]
            wp_t = []
            for h in range(HPG):
                t = pW.tile([128, DIM], bf16, tag=f"wp{h}", name=f"wp{h}")
                nc.sync.dma_start(t[0:64, :], wpT[64 * h : 64 * (h + 1), :])
                wp_t.append(t)
            ones_t = pW.tile([128, 64], bf16, tag="ones")
            nc.gpsimd.memset(ones_t[:], 1.0)

            # ------------- projections (all outputs at partitions 0-63) --
            # qk[h] = [Qt_h; Kt_h] packed? No: one tile per head per kind,
            # data at partitions 0-63 so full-mode K=64 matmuls can read it.
            qt = [pP.tile([128, N], bf16, tag=f"qt{h}", name=f"qt{h}") for h in range(3)]
            kt = [pP.tile([128, N], bf16, tag=f"kt{h}", name=f"kt{h}") for h in range(3)]

            def qk_group(qb, wc, xt, dst0, dst1):
                qs = slice(512 * qb, 512 * (qb + 1))
                ps = psO.tile([128, 512], f32, tag="o", name="ps_qk")
                for k in range(KC):
                    nc.tensor.matmul(
                        ps[:],
                        lhsT=wqk_t[k][:, wc : wc + 128],
                        rhs=xt[k][:, qs],
                        start=(k == 0),
                        stop=(k == KC - 1),
                    )
                nc.vector.tensor_copy(dst0[0:64, qs], ps[0:64, :])
                nc.vector.tensor_copy(dst1[64:128, qs], ps[64:128, :])
                # head-1 rows also needed at partitions 0-63 (PV rhs is full
                # 128 but S row-tile T8 reads 64-127; out-proj needs 0-63):
                # actually S T8 reads 64-127 directly; no shift needed here.

            def g3_group(qb):
                qs = slice(512 * qb, 512 * (qb + 1))
                ps = psO.tile([128, 512], f32, tag="o", name="ps_g3")
                for k in range(KC):
                    nc.tensor.matmul(
                        ps[0:64, :],
                        lhsT=wqk_t[k][:, 128:192],
                        rhs=xq_t[k][:, qs],
                        start=(k == 0),
                        stop=(k == KC - 1),
                        tile_position=(0, 0),
                        skip_group_check=True,
                    )
                    nc.tensor.matmul(
                        ps[64:128, :],
                        lhsT=wqk_t[k][:, 320:384],
                        rhs=xk_t[k][:, qs],
                        start=(k == 0),
                        stop=(k == KC - 1),
                        tile_position=(0, 64),
                        skip_group_check=True,
                    )
                nc.vector.tensor_copy(qt[2][0:64, qs], ps[0:64, :])
                nc.vector.tensor_copy(kt[2][64:128, qs], ps[64:128, :])
                nc.gpsimd.dma_start(kt[2][0:64, qs], kt[2][64:128, qs])
                nc.gpsimd.dma_start(qt[2][64:128, qs], qt[2][0:64, qs])

            v_t = []
            for sc in range(SEQC):
                v_t.append(
                    pP.tile([128, 3 * 65], bf16, tag=f"v{sc}", name=f"v{sc}")
                )

            def v_group(sc):
                ps = psO.tile([128, 512], f32, tag="o", name="ps_v")
                for k in range(KC):
                    nc.tensor.matmul(
                        ps[:, 0:192],
                        lhsT=xv_t[k][:, 128 * sc : 128 * (sc + 1)],
                        rhs=wv_t[k][:],
                        start=(k == 0),
                        stop=(k == KC - 1),
                    )
                vt = v_t[sc]
                for h in range(HPG):
                    nc.vector.tensor_copy(
                        vt[:, 65 * h : 65 * h + 64], ps[:, 64 * h : 64 * (h + 1)]
                    )
                    nc.gpsimd.memset(vt[:, 65 * h + 64 : 65 * h + 65], 1.0)

            # K-side first: attention S needs all of kt before step 0;
            # V before g3 (V is consumed from attention step 1, g3 at the
            # first U1 window ~16 steps in).
            for qb in range(QB):
                qk_group(qb, 192, xk_t, kt[0], kt[1])
            for qb in range(QB):
                qk_group(qb, 0, xq_t, qt[0], qt[1])
            for qb in range(QB):
                g3_group(qb)
            for sc in range(SEQC):
                v_group(sc)

            # ---------------- attention ----------------
            otn = []
            for h in range(HPG):
                otn.append(pP.tile([128, N], bf16, tag=f"otn{h}", name=f"otn{h}"))
            tstash = []
            for i in range(12):
                tstash.append(
                    pSm.tile([65, 512], bf16, tag=f"tmp{i}", bufs=1, name=f"tmp{i}")
                )
            # denominators packed partition-major: head (qb,h) occupies a
            # [128, 4] column block; reciprocal then costs ~12 free-elems.
            denP = pSm.tile([128, 48], f32, tag="denP", bufs=1, name="denP")
            recP = pSm.tile([128, 48], bf16, tag="recP", bufs=1, name="recP")
            gstep = [0]
            bg = deque()  # entries: (earliest_step, closure)


            def pump(k):
                for _ in range(k):
                    if bg and bg[0][0] <= gstep[0]:
                        bg.popleft()[1]()

            def collect(h, qb, o_ps):
                idx = 3 * qb + h
                nc.vector.tensor_copy(tstash[idx][:], o_ps[0:65, :])
                # denominator stays f32: DMA the psum row is impossible, so
                # re-read the bf16 stash row would lose bits; instead copy
                # the f32 denom row via ScalarE into the packed tile? DMA
                # can't read PSUM. Keep a small f32 row copy on DVE:
                nc.gpsimd.dma_start(
                    denP[:, 4 * idx : 4 * idx + 4],
                    tstash[idx][64:65, :],
                )

            def push_finalize_outproj(qb):
                qs = slice(512 * qb, 512 * (qb + 1))
                st = {}
                s0 = gstep[0]

                def f_recip():
                    cs = slice(12 * qb, 12 * qb + 12)
                    with nc.allow_low_precision(reason="softmax denom bcast"):
                        nc.vector.reciprocal(recP[:, cs], denP[:, cs])

                bg.append((s0 + 1, f_recip))

                def mk_head(h):
                    def f_head():
                        idx = 3 * qb + h
                        r1 = pSm.tile([1, 512], bf16, tag="r1", name="r1")
                        nc.gpsimd.dma_start(
                            r1[:], recP[:, 4 * idx : 4 * idx + 4]
                        )
                        rbs = pSm.tile([64, 512], bf16, tag="rbs", name="rbs")
                        nc.gpsimd.partition_broadcast(rbs[:], r1[0:1, :])
                        for j in range(4):
                            js = slice(128 * j, 128 * (j + 1))
                            gs = slice(512 * qb + 128 * j, 512 * qb + 128 * (j + 1))
                            nc.vector.tensor_mul(
                                otn[h][0:64, gs], tstash[idx][0:64, js], rbs[:, js]
                            )
                            if h == 1:
                                nc.gpsimd.dma_start(
                                    otn[0][64:128, gs], otn[1][0:64, gs]
                                )

                    return f_head

                for h in range(HPG):
                    bg.append((s0 + 6 + 2 * h, mk_head(h)))

                def mk_mm(qc, off, w, first, K128):
                    def f_mm():
                        if first:
                            st[qc] = (
                                psO.tile([128, 512], f32, tag="o", name="y1"),
                                pY.tile([128, DIM], f32, tag="y", name="ysb"),
                            )
                        y_ps, _ = st[qc]
                        cs = slice(128 * qc, 128 * (qc + 1))
                        if K128:
                            nc.tensor.matmul(
                                y_ps[:, 0:w],
                                lhsT=otn[0][:, cs],
                                rhs=wp_t[0][:, off : off + w],
                                start=True,
                                stop=False,
                            )
                        else:
                            nc.tensor.matmul(
                                y_ps[:, 0:w],
                                lhsT=otn[2][0:64, cs],
                                rhs=wp_t[2][0:64, off : off + w],
                                start=False,
                                stop=True,
                            )

                    return f_mm

                def mk_evac(qc, off, w, last):
                    def f_evac():
                        y_ps, y_sb = st[qc]
                        nc.vector.tensor_copy(y_sb[:, off : off + w], y_ps[:, 0:w])
                        if last:
                            cs = slice(128 * qc, 128 * (qc + 1))
                            nc.sync.dma_start(out[cs, :], y_sb[:])

                    return f_evac

                d = s0 + 13
                for qc in range(4 * qb, 4 * qb + 4):
                    bg.append((d, mk_mm(qc, 0, 512, True, True)))
                    bg.append((d, mk_mm(qc, 0, 512, False, False)))
                    bg.append((d + 1, mk_evac(qc, 0, 512, False)))
                    bg.append((d + 1, mk_mm(qc, 512, 256, False, True)))
                    bg.append((d + 2, mk_mm(qc, 512, 256, False, False)))
                    bg.append((d + 2, mk_evac(qc, 512, 256, True)))
                    d += 2

            # wp_t[0] must hold heads 0 and 1 stacked for the K=128 path
            nc.sync.dma_start(wp_t[0][64:128, :], wpT[64:128, :])

            pend = None

            def emit_pv(et, u, qb, step, o_ps):
                for c in (0, 1):
                    cs = slice(512 * c, 512 * (c + 1))
                    if u == 0:
                        hh, kc, oi = c, step, c
                    else:
                        hh, kc, oi = 2, 2 * step + c, 0
                    vsl = slice(65 * hh, 65 * hh + 65)
                    nc.tensor.matmul(
                        o_ps[oi][0:65, :],
                        lhsT=v_t[kc][:, vsl],
                        rhs=et[:, cs],
                        start=(kc == 0),
                        stop=(kc == SEQC - 1),
                    )

            windows = []
            for qb in range(QB):
                windows.append((0, qb))
                windows.append((1, qb))

            for u, qb in windows:
                qs = slice(512 * qb, 512 * (qb + 1))
                o_ps = {}
                for i in range(2 if u == 0 else 1):
                    o_ps[i] = psO.tile([128, 512], f32, tag="o", name=f"o{i}")
                nsteps = SEQC if u == 0 else SEQC // 2
                for step in range(nsteps):
                    s2 = psS2.tile([128, 1024], f32, tag="s2", name="s2")
                    if u == 0:
                        # S row-tile pair: head0 on rows 0-63, head1 on 64-127
                        nc.tensor.matmul(
                            s2[:, 0:512],
                            lhsT=kt[0][0:64, 128 * step : 128 * (step + 1)],
                            rhs=qt[0][0:64, qs],
                            start=True,
                            stop=True,
                            tile_position=(0, 0),
                            skip_group_check=True,
                        )
                        nc.tensor.matmul(
                            s2[:, 512:1024],
                            lhsT=kt[1][64:128, 128 * step : 128 * (step + 1)],
                            rhs=qt[1][64:128, qs],
                            start=True,
                            stop=True,
                            tile_position=(64, 0),
                            skip_group_check=True,
                        )
                    else:
                        k0, k1 = 2 * step, 2 * step + 1
                        nc.tensor.matmul(
                            s2[:, 0:512],
                            lhsT=kt[2][0:64, 128 * k0 : 128 * (k0 + 1)],
                            rhs=qt[2][0:64, qs],
                            start=True,
                            stop=True,
                            tile_position=(0, 0),
                            skip_group_check=True,
                        )
                        nc.tensor.matmul(
                            s2[:, 512:1024],
                            lhsT=kt[2][64:128, 128 * k1 : 128 * (k1 + 1)],
                            rhs=qt[2][64:128, qs],
                            start=True,
                            stop=True,
                            tile_position=(64, 0),
                            skip_group_check=True,
                        )
                    et = pEt.tile([128, 1024], bf16, tag="et", name="et")
                    nc.scalar.activation(et[:], s2[:], AF.Exp, scale=SCALE)
                    if pend is not None:
                        emit_pv(*pend)
                        if pend[3] == (SEQC if pend[1] == 0 else SEQC // 2) - 1:
                            pu, pqb, po = pend[1], pend[2], pend[4]
                            if pu == 0:
                                collect(0, pqb, po[0])
                                collect(1, pqb, po[1])
                            else:
                                collect(2, pqb, po[0])
                                push_finalize_outproj(pqb)
                    pend = (et, u, qb, step, o_ps)
                    gstep[0] += 1
                    pump(2 if len(bg) > 16 else 1)
            emit_pv(*pend)
            collect(2, QB - 1, pend[4][0])
            push_finalize_outproj(QB - 1)
            gstep[0] += 100000
            pump(len(bg))

    nc.compile()
    return nc


def _shard_inputs(xq, xk, xv, Wq, Wk, Wv, Wp):
    bf = ml_dtypes.bfloat16
    in_maps = []
    xT = {}
    for b in range(B):
        xT[b] = tuple(
            np.ascontiguousarray(a[b].T).astype(bf) for a in (xq, xk, xv)
        )
    for c in range(NCORES):
        b, g = c // 4, c % 4
        hs = slice(192 * g, 192 * (g + 1))
        wqk = np.concatenate([Wq[hs].T, Wk[hs].T], axis=1)
        in_maps.append(
            {
                "xqT": xT[b][0],
                "xkT": xT[b][1],
                "xvT": xT[b][2],
                "wqkT": np.ascontiguousarray(wqk).astype(bf),
                "wvT": np.ascontiguousarray(Wv[hs].T).astype(bf),
                "wpT": np.ascontiguousarray(Wp[:, hs].T).astype(bf),
            }
        )
    return in_maps


def _ensure_ntff_hook():
    """Register the axon NTFF profiling hook if the stub antenv lacks it."""
    import types

    try:
        from antenv.axon_hooks import get_axon_ntff_profile_hook  # noqa: F401

        return
    except ImportError:
        pass
    try:
        import antenv
        from trn_agent_boot.trn_boot import _ntff_profile_via_ctypes

        so_path = "/opt/axon/libaxon_pjrt.so"
        hook = _ntff_profile_via_ctypes(so_path) if os.path.exists(so_path) else None
        mod = types.ModuleType("antenv.axon_hooks")
        _state = {"h": hook}
        mod.get_axon_ntff_profile_hook = lambda: _state["h"]
        mod.set_axon_ntff_profile_hook = lambda h: _state.__setitem__("h", h)
        sys.modules["antenv.axon_hooks"] = mod
        antenv.axon_hooks = mod
    except Exception:
        pass


def kernel(xq, xk, xv, Wq, Wk, Wv, Wp, bp):
    global LAST_RESULT
    from concourse.bass_utils import run_bass_kernel_spmd

    key = "nc"
    if key not in _BUILT:
        _BUILT[key] = build_bass()
    nc = _BUILT[key]

    xq, xk, xv = (np.asarray(a, np.float32) for a in (xq, xk, xv))
    Wq, Wk, Wv, Wp = (np.asarray(a, np.float32) for a in (Wq, Wk, Wv, Wp))
    bp = np.asarray(bp, np.float32)

    in_maps = _shard_inputs(xq, xk, xv, Wq, Wk, Wv, Wp)
    trace = bool(os.environ.get("BASS_KERNEL_TRACE"))
    if trace:
        _ensure_ntff_hook()
    res = run_bass_kernel_spmd(
        nc, in_maps, core_ids=list(range(NCORES)), trace=trace
    )
    LAST_RESULT = res
    parts = [res.results[i]["out"].astype(np.float32) for i in range(NCORES)]
    out = np.stack(
        [
            parts[0] + parts[1] + parts[2] + parts[3],
            parts[4] + parts[5] + parts[6] + parts[7],
        ]
    )
    return (out + bp[None, None, :]).astype(np.float32)


# revision 29
# speedup vs baseline: 1.1142x; 1.0633x over previous
"""Self-contained Trainium2 Bass kernel for 12-head attention.

Module: out = softmax((xq Wq^T)(xk Wk^T)^T / sqrt(64)) (xv Wv^T) Wp^T + bp
Shapes: xq/xk/xv [2, 2048, 768]; W* [768, 768]; bp [768].

Sharding (8 cores): core c handles batch b = c//4 and head group g = c%4
(3 of the 12 heads).  Each core computes its heads' attention plus the
partial output projection (contraction over its 192 feature columns of
Wp).  Host unshard: out[b] = sum of the 4 group partials + bias.

Per-core dataflow (all matmul operands bf16, fp32 PSUM accumulation):
  Qt/Kt [64, 2048] transposed layout, head pairs packed into 128
  partitions; V [2048, 64] natural + ones column (denominator trick).
  S^T[k,q] = Kt^T Qt via 64-row PE array tiling (two concurrent tiles);
  Et = exp(S*scale) on ScalarE (|S*scale| <= ~3, no max needed);
  Ot[65,q] = V_aug^T Et row-tiled into two PSUM banks; normalize via
  reciprocal + rank-1 PE broadcast; Y[q,768] = OtN^T WpT per q-chunk.
"""

import os
import sys

import numpy as np

for _p in ("/opt/trn_rl_repo",):
    if _p not in sys.path and os.path.isdir(_p):
        sys.path.insert(0, _p)

import ml_dtypes

DIM = 768
NH = 12
HD = 64
N = 2048
B = 2
SCALE = HD ** -0.5
NCORES = 8
HPG = 3  # heads per group (core)

_BUILT = {}
LAST_RESULT = None


def build_bass():
    import concourse.bacc as bacc
    import concourse.mybir as mybir
    import concourse.tile as tile

    bf16 = mybir.dt.bfloat16
    f32 = mybir.dt.float32
    AF = mybir.ActivationFunctionType

    nc = bacc.Bacc("TRN2", target_bir_lowering=False, debug=False)
    xqT = nc.declare_dram_parameter("xqT", [DIM, N], bf16, isOutput=False)
    xkT = nc.declare_dram_parameter("xkT", [DIM, N], bf16, isOutput=False)
    xvT = nc.declare_dram_parameter("xvT", [DIM, N], bf16, isOutput=False)
    wqkT = nc.declare_dram_parameter("wqkT", [DIM, 2 * 192], bf16, isOutput=False)
    wvT = nc.declare_dram_parameter("wvT", [DIM, 192], bf16, isOutput=False)
    wpT = nc.declare_dram_parameter("wpT", [192, DIM], bf16, isOutput=False)
    out = nc.declare_dram_parameter("out", [N, DIM], f32, isOutput=True)

    KC = DIM // 128  # 6 contraction chunks for projections
    QB = N // 512    # 4 query blocks
    SEQC = N // 128  # 16 sequence chunks

    with tile.TileContext(nc) as tc:
        from collections import deque
        from contextlib import ExitStack

        with ExitStack() as ctx:
            pX = ctx.enter_context(tc.tile_pool(name="px", bufs=1))
            pW = ctx.enter_context(tc.tile_pool(name="pw", bufs=1))
            pP = ctx.enter_context(tc.tile_pool(name="pp", bufs=1))
            pEt = ctx.enter_context(tc.tile_pool(name="pet", bufs=6))
            pSm = ctx.enter_context(tc.tile_pool(name="psm", bufs=4))
            pY = ctx.enter_context(tc.tile_pool(name="py", bufs=3))
            psS2 = ctx.enter_context(tc.tile_pool(name="pss2", bufs=2, space="PSUM"))
            psO = ctx.enter_context(tc.tile_pool(name="pso", bufs=4, space="PSUM"))

            # ---------------- DMA inputs (first-needed first) ------------
            tiles = {}
            for nm, src_t, w in (
                ("wqk", wqkT, 384),
                ("xk", xkT, N),
                ("xq", xqT, N),
                ("wv", wvT, 192),
                ("xv", xvT, N),
            ):
                for k in range(KC):
                    t = pX.tile([128, w], bf16, tag=f"{nm}{k}", name=f"{nm}{k}")
                    nc.sync.dma_start(t[:], src_t[128 * k : 128 * (k + 1), :])
                    tiles[nm, k] = t
            xq_t = [tiles["xq", k] for k in range(KC)]
            xk_t = [tiles["xk", k] for k in range(KC)]
            xv_t = [tiles["xv", k] for k in range(KC)]
            wqk_t = [tiles["wqk", k] for k in range(KC)]
            wv_t = [tiles["wv", k] for k in range(KC)]
            wp_t = []
            for h in range(HPG):
                t = pW.tile([128, DIM], bf16, tag=f"wp{h}", name=f"wp{h}")
                nc.sync.dma_start(t[0:64, :], wpT[64 * h : 64 * (h + 1), :])
                wp_t.append(t)
            ones_t = pW.tile([128, 64], bf16, tag="ones")
            nc.gpsimd.memset(ones_t[:], 1.0)

            # ------------- projections (all outputs at partitions 0-63) --
            # qk[h] = [Qt_h; Kt_h] packed? No: one tile per head per kind,
            # data at partitions 0-63 so full-mode K=64 matmuls can read it.
            qt = [pP.tile([128, N], bf16, tag=f"qt{h}", name=f"qt{h}") for h in range(3)]
            kt = [pP.tile([128, N], bf16, tag=f"kt{h}", name=f"kt{h}") for h in range(3)]

            def qk_group(qb, wc, xt, dst0, dst1):
                qs = slice(512 * qb, 512 * (qb + 1))
                ps = psO.tile([128, 512], f32, tag="o", name="ps_qk")
                for k in range(KC):
                    nc.tensor.matmul(
                        ps[:],
                        lhsT=wqk_t[k][:, wc : wc + 128],
                        rhs=xt[k][:, qs],
                        start=(k == 0),
                        stop=(k == KC - 1),
                    )
                nc.vector.tensor_copy(dst0[0:64, qs], ps[0:64, :])
                nc.vector.tensor_copy(dst1[64:128, qs], ps[64:128, :])
                # head-1 rows also needed at partitions 0-63 (PV rhs is full
                # 128 but S row-tile T8 reads 64-127; out-proj needs 0-63):
                # actually S T8 reads 64-127 directly; no shift needed here.

            def g3_group(qb):
                qs = slice(512 * qb, 512 * (qb + 1))
                ps = psO.tile([128, 512], f32, tag="o", name="ps_g3")
                for k in range(KC):
                    nc.tensor.matmul(
                        ps[0:64, :],
                        lhsT=wqk_t[k][:, 128:192],
                        rhs=xq_t[k][:, qs],
                        start=(k == 0),
                        stop=(k == KC - 1),
                        tile_position=(0, 0),
                        skip_group_check=True,
                    )
                    nc.tensor.matmul(
                        ps[64:128, :],
                        lhsT=wqk_t[k][:, 320:384],
                        rhs=xk_t[k][:, qs],
                        start=(k == 0),
                        stop=(k == KC - 1),
                        tile_position=(0, 64),
                        skip_group_check=True,
                    )
                nc.vector.tensor_copy(qt[2][0:64, qs], ps[0:64, :])
                nc.vector.tensor_copy(kt[2][64:128, qs], ps[64:128, :])
                nc.gpsimd.dma_start(kt[2][0:64, qs], kt[2][64:128, qs])
                nc.gpsimd.dma_start(qt[2][64:128, qs], qt[2][0:64, qs])

            v_t = []
            for sc in range(SEQC):
                v_t.append(
                    pP.tile([128, 3 * 65], bf16, tag=f"v{sc}", name=f"v{sc}")
                )

            def v_group(sc):
                ps = psO.tile([128, 512], f32, tag="o", name="ps_v")
                for k in range(KC):
                    nc.tensor.matmul(
                        ps[:, 0:192],
                        lhsT=xv_t[k][:, 128 * sc : 128 * (sc + 1)],
                        rhs=wv_t[k][:],
                        start=(k == 0),
                        stop=(k == KC - 1),
                    )
                vt = v_t[sc]
                for h in range(HPG):
                    nc.vector.tensor_copy(
                        vt[:, 65 * h : 65 * h + 64], ps[:, 64 * h : 64 * (h + 1)]
                    )
                    nc.gpsimd.memset(vt[:, 65 * h + 64 : 65 * h + 65], 1.0)

            # K-side first: attention S needs all of kt before step 0;
            # V before g3 (V is consumed from attention step 1, g3 at the
            # first U1 window ~16 steps in).
            for qb in range(QB):
                qk_group(qb, 192, xk_t, kt[0], kt[1])
            for qb in range(QB):
                qk_group(qb, 0, xq_t, qt[0], qt[1])
            for qb in range(QB):
                g3_group(qb)
            for sc in range(SEQC):
                v_group(sc)

            # ---------------- attention ----------------
            otn = []
            for h in range(HPG):
                otn.append(pP.tile([128, N], bf16, tag=f"otn{h}", name=f"otn{h}"))
            tstash = []
            for i in range(12):
                tstash.append(
                    pSm.tile([65, 512], bf16, tag=f"tmp{i}", bufs=1, name=f"tmp{i}")
                )
            # denominators packed partition-major: head (qb,h) occupies a
            # [128, 4] column block; reciprocal then costs ~12 free-elems.
            denP = pSm.tile([128, 48], f32, tag="denP", bufs=1, name="denP")
            recP = pSm.tile([128, 48], bf16, tag="recP", bufs=1, name="recP")
            gstep = [0]
            bg = deque()  # entries: (earliest_step, closure)


            def pump(k):
                for _ in range(k):
                    if bg and bg[0][0] <= gstep[0]:
                        bg.popleft()[1]()

            def collect(h, qb, o_ps):
                idx = 3 * qb + h
                nc.vector.tensor_copy(tstash[idx][:], o_ps[0:65, :])
                # denominator stays f32: DMA the psum row is impossible, so
                # re-read the bf16 stash row would lose bits; instead copy
                # the f32 denom row via ScalarE into the packed tile? DMA
                # can't read PSUM. Keep a small f32 row copy on DVE:
                nc.gpsimd.dma_start(
                    denP[:, 4 * idx : 4 * idx + 4],
                    tstash[idx][64:65, :],
                )

            def push_finalize_outproj(qb):
                qs = slice(512 * qb, 512 * (qb + 1))
                st = {}
                s0 = gstep[0]

                def f_recip():
                    cs = slice(12 * qb, 12 * qb + 12)
                    with nc.allow_low_precision(reason="softmax denom bcast"):
                        nc.vector.reciprocal(recP[:, cs], denP[:, cs])

                bg.append((s0 + 1, f_recip))

                def mk_head(h):
                    def f_head():
                        idx = 3 * qb + h
                        r1 = pSm.tile([1, 512], bf16, tag="r1", name="r1")
                        nc.gpsimd.dma_start(
                            r1[:], recP[:, 4 * idx : 4 * idx + 4]
                        )
                        rbs = pSm.tile([64, 512], bf16, tag="rbs", name="rbs")
                        nc.gpsimd.partition_broadcast(rbs[:], r1[0:1, :])
                        nc.vector.tensor_mul(
                            otn[h][0:64, qs], tstash[idx][0:64, :], rbs[:]
                        )
                        if h == 1:
                            # pack head1 next to head0 for a K=128 out-proj
                            nc.gpsimd.dma_start(
                                otn[0][64:128, qs], otn[1][0:64, qs]
                            )

                    return f_head

                for h in range(HPG):
                    bg.append((s0 + 6 + 2 * h, mk_head(h)))

                def mk_mm(qc, off, w, first, K128):
                    def f_mm():
                        if first:
                            st[qc] = (
                                psO.tile([128, 512], f32, tag="o", name="y1"),
                                pY.tile([128, DIM], f32, tag="y", name="ysb"),
                            )
                        y_ps, _ = st[qc]
                        cs = slice(128 * qc, 128 * (qc + 1))
                        if K128:
                            nc.tensor.matmul(
                                y_ps[:, 0:w],
                                lhsT=otn[0][:, cs],
                                rhs=wp_t[0][:, off : off + w],
                                start=True,
                                stop=False,
                            )
                        else:
                            nc.tensor.matmul(
                                y_ps[:, 0:w],
                                lhsT=otn[2][0:64, cs],
                                rhs=wp_t[2][0:64, off : off + w],
                                start=False,
                                stop=True,
                            )

                    return f_mm

                def mk_evac(qc, off, w, last):
                    def f_evac():
                        y_ps, y_sb = st[qc]
                        nc.vector.tensor_copy(y_sb[:, off : off + w], y_ps[:, 0:w])
                        if last:
                            cs = slice(128 * qc, 128 * (qc + 1))
                            nc.sync.dma_start(out[cs, :], y_sb[:])

                    return f_evac

                d = s0 + 13
                for qc in range(4 * qb, 4 * qb + 4):
                    bg.append((d, mk_mm(qc, 0, 512, True, True)))
                    bg.append((d, mk_mm(qc, 0, 512, False, False)))
                    bg.append((d + 1, mk_evac(qc, 0, 512, False)))
                    bg.append((d + 1, mk_mm(qc, 512, 256, False, True)))
                    bg.append((d + 2, mk_mm(qc, 512, 256, False, False)))
                    bg.append((d + 2, mk_evac(qc, 512, 256, True)))
                    d += 2

            # wp_t[0] must hold heads 0 and 1 stacked for the K=128 path
            nc.sync.dma_start(wp_t[0][64:128, :], wpT[64:128, :])

            pend = None

            def emit_pv(et, u, qb, step, o_ps):
                for c in (0, 1):
                    cs = slice(512 * c, 512 * (c + 1))
                    if u == 0:
                        hh, kc, oi = c, step, c
                    else:
                        hh, kc, oi = 2, 2 * step + c, 0
                    vsl = slice(65 * hh, 65 * hh + 65)
                    nc.tensor.matmul(
                        o_ps[oi][0:65, :],
                        lhsT=v_t[kc][:, vsl],
                        rhs=et[:, cs],
                        start=(kc == 0),
                        stop=(kc == SEQC - 1),
                    )

            windows = []
            for qb in range(QB):
                windows.append((0, qb))
                windows.append((1, qb))

            for u, qb in windows:
                qs = slice(512 * qb, 512 * (qb + 1))
                o_ps = {}
                for i in range(2 if u == 0 else 1):
                    o_ps[i] = psO.tile([128, 512], f32, tag="o", name=f"o{i}")
                nsteps = SEQC if u == 0 else SEQC // 2
                for step in range(nsteps):
                    s2 = psS2.tile([128, 1024], f32, tag="s2", name="s2")
                    if u == 0:
                        # S row-tile pair: head0 on rows 0-63, head1 on 64-127
                        nc.tensor.matmul(
                            s2[:, 0:512],
                            lhsT=kt[0][0:64, 128 * step : 128 * (step + 1)],
                            rhs=qt[0][0:64, qs],
                            start=True,
                            stop=True,
                            tile_position=(0, 0),
                            skip_group_check=True,
                        )
                        nc.tensor.matmul(
                            s2[:, 512:1024],
                            lhsT=kt[1][64:128, 128 * step : 128 * (step + 1)],
                            rhs=qt[1][64:128, qs],
                            start=True,
                            stop=True,
                            tile_position=(64, 0),
                            skip_group_check=True,
                        )
                    else:
                        k0, k1 = 2 * step, 2 * step + 1
                        nc.tensor.matmul(
                            s2[:, 0:512],
                            lhsT=kt[2][0:64, 128 * k0 : 128 * (k0 + 1)],
                            rhs=qt[2][0:64, qs],
                            start=True,
                            stop=True,
                            tile_position=(0, 0),
                            skip_group_check=True,
                        )
                        nc.tensor.matmul(
                            s2[:, 512:1024],
                            lhsT=kt[2][64:128, 128 * k1 : 128 * (k1 + 1)],
                            rhs=qt[2][64:128, qs],
                            start=True,
                            stop=True,
                            tile_position=(64, 0),
                            skip_group_check=True,
                        )
                    et = pEt.tile([128, 1024], bf16, tag="et", name="et")
                    nc.scalar.activation(et[:], s2[:], AF.Exp, scale=SCALE)
                    if pend is not None:
                        emit_pv(*pend)
                        if pend[3] == (SEQC if pend[1] == 0 else SEQC // 2) - 1:
                            pu, pqb, po = pend[1], pend[2], pend[4]
                            if pu == 0:
                                collect(0, pqb, po[0])
                                collect(1, pqb, po[1])
                            else:
                                collect(2, pqb, po[0])
                                push_finalize_outproj(pqb)
                    pend = (et, u, qb, step, o_ps)
                    gstep[0] += 1
                    pump(2 if len(bg) > 16 else 1)
            emit_pv(*pend)
            collect(2, QB - 1, pend[4][0])
            push_finalize_outproj(QB - 1)
            gstep[0] += 100000
            pump(len(bg))

    nc.compile()
    return nc


def _shard_inputs(xq, xk, xv, Wq, Wk, Wv, Wp):
    bf = ml_dtypes.bfloat16
    in_maps = []
    xT = {}
    for b in range(B):
        xT[b] = tuple(
            np.ascontiguousarray(a[b].T).astype(bf) for a in (xq, xk, xv)
        )
    for c in range(NCORES):
        b, g = c // 4, c % 4
        hs = slice(192 * g, 192 * (g + 1))
        wqk = np.concatenate([Wq[hs].T, Wk[hs].T], axis=1)
        in_maps.append(
            {
                "xqT": xT[b][0],
                "xkT": xT[b][1],
                "xvT": xT[b][2],
                "wqkT": np.ascontiguousarray(wqk).astype(bf),
                "wvT": np.ascontiguousarray(Wv[hs].T).astype(bf),
                "wpT": np.ascontiguousarray(Wp[:, hs].T).astype(bf),
            }
        )
    return in_maps


def _ensure_ntff_hook():
    """Register the axon NTFF profiling hook if the stub antenv lacks it."""
    import types

    try:
        from antenv.axon_hooks import get_axon_ntff_profile_hook  # noqa: F401

        return
    except ImportError:
        pass
    try:
        import antenv
        from trn_agent_boot.trn_boot import _ntff_profile_via_ctypes

        so_path = "/opt/axon/libaxon_pjrt.so"
        hook = _ntff_profile_via_ctypes(so_path) if os.path.exists(so_path) else None
        mod = types.ModuleType("antenv.axon_hooks")
        _state = {"h": hook}
        mod.get_axon_ntff_profile_hook = lambda: _state["h"]
        mod.set_axon_ntff_profile_hook = lambda h: _state.__setitem__("h", h)
        sys.modules["antenv.axon_hooks"] = mod
        antenv.axon_hooks = mod
    except Exception:
        pass


def kernel(xq, xk, xv, Wq, Wk, Wv, Wp, bp):
    global LAST_RESULT
    from concourse.bass_utils import run_bass_kernel_spmd

    key = "nc"
    if key not in _BUILT:
        _BUILT[key] = build_bass()
    nc = _BUILT[key]

    xq, xk, xv = (np.asarray(a, np.float32) for a in (xq, xk, xv))
    Wq, Wk, Wv, Wp = (np.asarray(a, np.float32) for a in (Wq, Wk, Wv, Wp))
    bp = np.asarray(bp, np.float32)

    in_maps = _shard_inputs(xq, xk, xv, Wq, Wk, Wv, Wp)
    trace = bool(os.environ.get("BASS_KERNEL_TRACE"))
    if trace:
        _ensure_ntff_hook()
    res = run_bass_kernel_spmd(
        nc, in_maps, core_ids=list(range(NCORES)), trace=trace
    )
    LAST_RESULT = res
    parts = [res.results[i]["out"].astype(np.float32) for i in range(NCORES)]
    out = np.stack(
        [
            parts[0] + parts[1] + parts[2] + parts[3],
            parts[4] + parts[5] + parts[6] + parts[7],
        ]
    )
    return (out + bp[None, None, :]).astype(np.float32)
